# revision 1
# baseline (speedup 1.0000x reference)
"""Trainium2 Bass kernel for nn_AliasFreeActivation (alias-free GAN activation).

Pipeline per (n, c) plane X [64, 64]:
    y = Wdn.T @ ( sqrt(2) * lrelu_0.2( Wup.T @ (X + b) @ Wup ) ) @ Wdn
where Wup [64, 128] / Wdn [128, 64] are the upfirdn band matrices for the
separable 12-tap filter (up=2 / down=2), built on host.

Device mapping (fused matmul chain — zero transposes):
    M1  t1 = x_aug.T @ Wup_aug    [2 planes: 128(2w), 128h']   (data as lhsT)
    M2  u^T = Wup.T @ t1_p        [128w', 128h']               (filter stationary)
    L   s = lrelu(sqrt2 * u)      ACT Lrelu alpha=0.2, PSUM->SBUF bf16
    M3  v = s.T @ Wdn             [128h', 64w'']               (data as lhsT)
    M4  y = Wdn.T @ v_all         [64h'', 512]                 (one matmul / group)

Sharding: pure data parallel over batch: core i gets input[i] -> [512, 64, 64].
Each core processes 64 groups of 8 channel-planes.
"""

import os
import sys

for _p in ("/opt/trn_rl_repo", "/opt/pypackages"):
    if _p not in sys.path:
        sys.path.append(_p)

import numpy as np
import ml_dtypes

N_CORES = 8
B, C, H, W = 8, 512, 64, 64
GROUP = 8                 # channel planes per group
N_GROUPS = C // GROUP     # 64
DMA_BATCH = int(os.environ.get("DMAB", 2))  # groups per DMA transfer
UP_LEN = 128
NEG_SLOPE = 0.2
SQRT2 = float(2.0 ** 0.5)

# 12-tap hann-windowed-sinc lowpass, as in the reference module
_FILT = np.array([0.0, 0.00398, -0.01884, -0.05155, 0.12443, 0.44197,
                  0.44197, 0.12443, -0.05155, -0.01884, 0.00398, 0.0],
                 dtype=np.float64)
_FILT = _FILT / _FILT.sum()

_BF16 = ml_dtypes.bfloat16

_LAST_RESULT = None   # BassKernelResults of the most recent run (for test.py)
_CACHED = None        # (nc, meta) cache so repeat kernel() calls skip rebuild


def _upfirdn_matrix(k, L, up, down, pad0, pad1):
    """Band matrix Wf such that y = x @ Wf applies upfirdn along an axis."""
    K = len(k)
    Ld = (L - 1) * up + 1
    n_out = (Ld + pad0 + (pad1 + up - 1) - K) // down + 1
    Wf = np.zeros((L, n_out), dtype=np.float64)
    for j in range(n_out):
        for t in range(K):
            m = j * down + t - pad0
            if 0 <= m < Ld and m % up == 0:
                Wf[m // up, j] += k[K - 1 - t]
    return Wf


def _build_consts(up_filter, down_filter):
    k_up = np.asarray(up_filter, dtype=np.float64) * 2.0   # prescaled by UP
    k_dn = np.asarray(down_filter, dtype=np.float64)
    Wup = _upfirdn_matrix(k_up, 64, 2, 1, 6, 5)            # [64, 128]
    Wdn = _upfirdn_matrix(k_dn, 128, 1, 2, 5, 5)           # [128, 64]
    # zero-padded K=128 variants: contract only the low/high 64 partitions
    # of a full-128 rhs (dodges the base-partition-64 operand restriction)
    wupz = np.zeros((128, 2 * UP_LEN), dtype=np.float64)
    wupz[0:64, 0:UP_LEN] = Wup        # "even" half: contracts partitions 0-63
    wupz[64:128, UP_LEN:] = Wup       # "odd" half: contracts partitions 64-127
    return Wup.astype(_BF16), Wdn.astype(_BF16), wupz.astype(_BF16)


def _build_bass(n_groups=N_GROUPS, repeat=1):
    import concourse.bacc as bacc
    import concourse.mybir as mybir
    from concourse.tile import TileContext

    f32 = mybir.dt.float32
    bf16 = mybir.dt.bfloat16

    nc = bacc.Bacc("TRN2", target_bir_lowering=False)

    # x / out live in DRAM as [H, C*W] (host pre/post-transposes) so every
    # DMA is a plain 2-D slice with 512*dsize contiguous bytes per partition.
    x = nc.dram_tensor("x", [H, C * W], bf16, kind="ExternalInput")
    wup = nc.dram_tensor("wup", [64, UP_LEN], bf16, kind="ExternalInput")
    wupz = nc.dram_tensor("wupz", [128, 2 * UP_LEN], bf16,
                          kind="ExternalInput")
    wdn = nc.dram_tensor("wdn", [UP_LEN, W], bf16, kind="ExternalInput")
    out = nc.dram_tensor("out", [H, C * W], f32, kind="ExternalOutput")

    with TileContext(nc) as tc:
        with (
            tc.tile_pool(name="consts", bufs=1) as cpool,
            tc.tile_pool(name="xt", bufs=int(os.environ.get("XB", 3))) as xpool,
            tc.tile_pool(name="t1ps", bufs=int(os.environ.get("T1B", 2)), space="PSUM") as t1ps_pool,
            tc.tile_pool(name="t1sb", bufs=int(os.environ.get("T1SB", 4))) as t1sb_pool,
            tc.tile_pool(name="ups", bufs=int(os.environ.get("UPB", 2)), space="PSUM") as ups_pool,
            tc.tile_pool(name="ssb", bufs=int(os.environ.get("SSB", 4))) as ssb_pool,
            tc.tile_pool(name="vps", bufs=int(os.environ.get("VPB", 1)), space="PSUM") as vps_pool,
            tc.tile_pool(name="vsb", bufs=int(os.environ.get("VSB", 4))) as vsb_pool,
            tc.tile_pool(name="yps", bufs=int(os.environ.get("YPB", 1)), space="PSUM") as yps_pool,
            tc.tile_pool(name="ysb", bufs=2) as ysb_pool,
        ):
            wup_sb = cpool.tile([64, UP_LEN], bf16)
            nc.sync.dma_start(out=wup_sb[:], in_=wup[:])
            wupz_sb = cpool.tile([128, 2 * UP_LEN], bf16)
            nc.sync.dma_start(out=wupz_sb[:], in_=wupz[:])
            wdn_sb = cpool.tile([UP_LEN, W], bf16)
            nc.sync.dma_start(out=wdn_sb[:], in_=wdn[:])
            alpha_sb = cpool.tile([128, 1], f32)
            nc.vector.memset(alpha_sb[:], NEG_SLOPE)

            assert n_groups % DMA_BATCH == 0 or n_groups < DMA_BATCH
            dma_b = min(DMA_BATCH, n_groups)
            xt4 = None
            ysb4 = None

            import contextlib
            rep_ctx = (tc.For_i(0, repeat, 1) if repeat > 1
                       else contextlib.nullcontext())
            with rep_ctx:
                _group_loop(nc, tc, mybir, n_groups, dma_b, locals())

    nc.compile()
    return nc


def _group_loop(nc, tc, mybir, n_groups, dma_b, env):
    f32 = mybir.dt.float32
    bf16 = mybir.dt.bfloat16
    x, out = env["x"], env["out"]
    wup_sb, wdn_sb, alpha_sb = env["wup_sb"], env["wdn_sb"], env["alpha_sb"]
    wupz_sb = env["wupz_sb"]
    xpool, t1ps_pool, t1sb_pool = env["xpool"], env["t1ps_pool"], env["t1sb_pool"]
    ups_pool, ssb_pool, vps_pool = env["ups_pool"], env["ssb_pool"], env["vps_pool"]
    vsb_pool, yps_pool, ysb_pool = env["vsb_pool"], env["yps_pool"], env["ysb_pool"]
    xt4 = None
    ysb4 = None
    if True:
            for g in range(n_groups):
                c0 = g * GROUP
                gb = g % dma_b
                # ---- load x for dma_b groups at once (bf16, host layout) ----
                if gb == 0:
                    xt4 = xpool.tile([64, dma_b * GROUP * W], bf16)
                    nc.sync.dma_start(
                        out=xt4[:],
                        in_=x[:, c0 * W:(c0 + dma_b * GROUP) * W])
                    ysb4 = ysb_pool.tile([64, dma_b * GROUP * W], f32)
                xt = xt4[:, gb * GROUP * W:(gb + 1) * GROUP * W]

                # ---- M1: 4 pair matmuls -> t1 pairs [128(2pl w), 128h'] ----
                # (full-128 output partitions; operands at base 0)
                t1ps = t1ps_pool.tile([128, 512], f32)
                for j in range(4):
                    nc.tensor.matmul(
                        t1ps[:, j * 128:(j + 1) * 128],
                        lhsT=xt[:, j * 128:(j + 1) * 128],
                        rhs=wup_sb[:],
                        start=True, stop=True,
                    )
                t1sb = t1sb_pool.tile([128, 512], bf16)
                nc.vector.tensor_copy(out=t1sb[:], in_=t1ps[:])

                # ---- M2: 2 bulk K=128 matmuls with zero-padded weights ----
                # even half contracts partitions 0-63 (even planes of each
                # pair), odd half contracts 64-127 — rhs stays at base 0.
                ups = ups_pool.tile([128, 1024], f32)
                for half in range(2):
                    nc.tensor.matmul(
                        ups[:, half * 512:(half + 1) * 512],
                        lhsT=wupz_sb[:, half * UP_LEN:(half + 1) * UP_LEN],
                        rhs=t1sb[:],
                        start=True, stop=True,
                    )

                # ---- L: lrelu evac PSUM->SBUF bf16 (one wide op) ----
                ssb = ssb_pool.tile([128, 1024], bf16)
                if os.environ.get("PSPLIT", "0") == "1":
                    for half in range(2):
                        nc.scalar.activation(
                            out=ssb[:, half * 512:(half + 1) * 512],
                            in_=ups[:, half * 512:(half + 1) * 512],
                            func=mybir.ActivationFunctionType.Prelu,
                            scale=SQRT2,
                            alpha=alpha_sb[:],
                        )
                else:
                    nc.scalar.activation(
                        out=ssb[:],
                        in_=ups[:],
                        func=mybir.ActivationFunctionType.Prelu,
                        scale=SQRT2,
                        alpha=alpha_sb[:],
                    )

                # ---- M3: 8 matmuls -> v per plane [128h', 64w''] ----
                # plane p lives at ssb[:, (p%2)*512 + (p//2)*128 : +128]
                vps = vps_pool.tile([128, 512], f32)
                for p in range(GROUP):
                    s_off = (p % 2) * 512 + (p // 2) * 128
                    nc.tensor.matmul(
                        vps[:, p * 64:(p + 1) * 64],
                        lhsT=ssb[:, s_off:s_off + 128],
                        rhs=wdn_sb[:],
                        start=True, stop=True,
                    )
                vsb = vsb_pool.tile([128, 512], bf16)
                if os.environ.get("SWAPVY", "0") == "1":
                    nc.scalar.activation(
                        out=vsb[:], in_=vps[:],
                        func=mybir.ActivationFunctionType.Copy)
                else:
                    nc.vector.tensor_copy(out=vsb[:], in_=vps[:])

                # ---- M4: one matmul -> y [64h'', 512] ----
                yps = yps_pool.tile([64, 512], f32)
                nc.tensor.matmul(
                    yps[:], lhsT=wdn_sb[:], rhs=vsb[:], start=True, stop=True,
                )
                if os.environ.get("SWAPVY", "0") == "1":
                    nc.vector.tensor_copy(
                        out=ysb4[:, gb * GROUP * W:(gb + 1) * GROUP * W],
                        in_=yps[:])
                else:
                    nc.scalar.activation(
                        out=ysb4[:, gb * GROUP * W:(gb + 1) * GROUP * W],
                        in_=yps[:],
                        func=mybir.ActivationFunctionType.Copy)

                # ---- store dma_b groups at once ----
                if gb == dma_b - 1:
                    nc.sync.dma_start(
                        out=out[:, (c0 - (dma_b - 1) * GROUP) * W:
                                (c0 + GROUP) * W],
                        in_=ysb4[:],
                    )


def kernel(input, bias, up_filter, down_filter):
    global _LAST_RESULT, _CACHED
    from concourse.bass_utils import run_bass_kernel_spmd

    input = np.asarray(input, dtype=np.float32)
    bias = np.asarray(bias, dtype=np.float32)
    if np.any(bias):
        input = input + bias.reshape(1, C, 1, 1)
    # [B, C, H, W] -> per-core [H, C*W] bf16
    x_t = np.ascontiguousarray(
        input.astype(_BF16).transpose(0, 2, 1, 3).reshape(B, H, C * W))

    if _CACHED is None:
        _CACHED = _build_bass()
    nc = _CACHED

    wup_m, wdn_m, wupz_m = _build_consts(up_filter, down_filter)

    in_maps = []
    for i in range(N_CORES):
        in_maps.append({
            "x": x_t[i],
            "wup": wup_m,
            "wupz": wupz_m,
            "wdn": wdn_m,
        })

    res = run_bass_kernel_spmd(nc, in_maps, core_ids=list(range(N_CORES)))
    _LAST_RESULT = res
    # per-core [H, C*W] f32 -> [B, C, H, W]
    y = np.stack([r["out"] for r in res.results], axis=0)
    return np.ascontiguousarray(
        y.reshape(B, H, C, W).transpose(0, 2, 1, 3))



# revision 7
# speedup vs baseline: 1.1563x; 1.1563x over previous
"""Trainium2 Bass kernel for nn_AliasFreeActivation (alias-free GAN activation).

Pipeline per (n, c) plane X [64, 64]:
    y = Wdn.T @ ( sqrt(2) * lrelu_0.2( Wup.T @ (X + b) @ Wup ) ) @ Wdn
where Wup [64, 128] / Wdn [128, 64] are the upfirdn band matrices for the
separable 12-tap filter (up=2 / down=2), built on host.

Device mapping (fused matmul chain, software-pipelined stages):
    A(g): DMA x, M1 (4 MMs, K=64): t1[(e,w), h'] -> DVE copy -> t1sb
    B(g): M2 (2 row-tiled MMs K=64 N=512): u[w', (p4,h')] -> ACT Prelu -> ssb
    C(g): M3 (8 MMs K=128 N=64): v[h', (c,w'')] -> DVE copy -> vsb
          M4 (2 col-tiled MMs K=128 M=64): y -> ACT copy -> ysb -> DMA out
Stages are emitted with skew (A(g+2), B(g+1), C(g)) so the cross-engine
chain M1 -> t1copy -> M2 -> Prelu -> M3 never stalls any engine.

Sharding: pure data parallel over batch: core i gets input[i] -> [512, 64, 64].
"""

import os
import sys

for _p in ("/opt/trn_rl_repo", "/opt/pypackages"):
    if _p not in sys.path:
        sys.path.append(_p)

import numpy as np
import ml_dtypes

N_CORES = 8
B, C, H, W = 8, 512, 64, 64
GROUP = 8                 # channel planes per group
N_GROUPS = C // GROUP     # 64
DMA_BATCH = int(os.environ.get("DMAB", 2))  # groups per DMA transfer
UP_LEN = 128
NEG_SLOPE = 0.2
SQRT2 = float(2.0 ** 0.5)

EVAC_T1 = os.environ.get("EVAC_T1", "vector")
EVAC_V = os.environ.get("EVAC_V", "vector")
EVAC_Y = os.environ.get("EVAC_Y", "act")
Y_COLTILE = os.environ.get("Y_COLTILE", "1") == "1"
M2_ROWTILE = os.environ.get("M2_ROWTILE", "1") == "1"

# 12-tap hann-windowed-sinc lowpass, as in the reference module
_FILT = np.array([0.0, 0.00398, -0.01884, -0.05155, 0.12443, 0.44197,
                  0.44197, 0.12443, -0.05155, -0.01884, 0.00398, 0.0],
                 dtype=np.float64)
_FILT = _FILT / _FILT.sum()

_BF16 = ml_dtypes.bfloat16

_LAST_RESULT = None   # BassKernelResults of the most recent run (for test.py)
_CACHED = None        # compiled nc cache so repeat kernel() calls skip rebuild


def _upfirdn_matrix(k, L, up, down, pad0, pad1):
    """Band matrix Wf such that y = x @ Wf applies upfirdn along an axis."""
    K = len(k)
    Ld = (L - 1) * up + 1
    n_out = (Ld + pad0 + (pad1 + up - 1) - K) // down + 1
    Wf = np.zeros((L, n_out), dtype=np.float64)
    for j in range(n_out):
        for t in range(K):
            m = j * down + t - pad0
            if 0 <= m < Ld and m % up == 0:
                Wf[m // up, j] += k[K - 1 - t]
    return Wf


def _build_consts(up_filter, down_filter):
    k_up = np.asarray(up_filter, dtype=np.float64) * 2.0   # prescaled by UP
    k_dn = np.asarray(down_filter, dtype=np.float64)
    Wup = _upfirdn_matrix(k_up, 64, 2, 1, 6, 5)            # [64, 128]
    Wdn = _upfirdn_matrix(k_dn, 128, 1, 2, 5, 5)           # [128, 64]
    wup2 = np.concatenate([Wup, Wup], axis=0)              # [128, 128] vstack
    wdn3 = Wdn * SQRT2                                     # sqrt2 folded here
    wdn2 = np.concatenate([Wdn, Wdn], axis=1)              # [128, 128] hstack
    return (wup2.astype(_BF16), wdn3.astype(_BF16), wdn2.astype(_BF16))


def _pack_x(x_core):
    """[C, H, W] f32 -> [64, C*W] bf16 device layout.

    x_dev[h, ((g*4 + p4)*2 + e)*64 + w] = x[g*8 + p4*2 + e, h, w]
    """
    xd = x_core.reshape(C, H, W).transpose(1, 0, 2)   # [h, c, w]
    return np.ascontiguousarray(xd.reshape(H, C * W).astype(_BF16))


def _unpack_y(y_dev):
    """[128, C*W/2] bf16 -> [C, H, W] f32.

    y_dev[ph*64 + h, (g*4 + pl)*64 + w] = y[g*8 + ph*4 + pl, h, w]
    """
    yg = y_dev.astype(np.float32).reshape(2, H, N_GROUPS, 4, W)  # [ph,h,g,pl,w]
    return np.ascontiguousarray(
        yg.transpose(2, 0, 3, 1, 4).reshape(C, H, W))


def _build_bass(n_groups=N_GROUPS, repeat=1):
    import concourse.bacc as bacc
    import concourse.mybir as mybir
    from concourse.tile import TileContext

    f32 = mybir.dt.float32
    bf16 = mybir.dt.bfloat16

    nc = bacc.Bacc("TRN2", target_bir_lowering=False)

    x = nc.dram_tensor("x", [H, n_groups * 8 * W], bf16, kind="ExternalInput")
    wup2 = nc.dram_tensor("wup2", [128, 128], bf16, kind="ExternalInput")
    wdn3 = nc.dram_tensor("wdn3", [UP_LEN, W], bf16, kind="ExternalInput")
    wdn2 = nc.dram_tensor("wdn2", [128, 128], bf16, kind="ExternalInput")
    out = nc.dram_tensor("out", [128, n_groups * 4 * W], bf16,
                         kind="ExternalOutput")

    dma_b = min(DMA_BATCH, n_groups)
    assert n_groups % dma_b == 0

    with TileContext(nc) as tc:
        with (
            tc.tile_pool(name="consts", bufs=1) as cpool,
            tc.tile_pool(name="xt", bufs=int(os.environ.get("XB", 3))) as xpool,
            tc.tile_pool(name="t1ps", bufs=int(os.environ.get("T1PB", 2)), space="PSUM") as t1ps_pool,
            tc.tile_pool(name="t1sb", bufs=int(os.environ.get("T1SB", 3))) as t1sb_pool,
            tc.tile_pool(name="ups", bufs=int(os.environ.get("UPB", 2)), space="PSUM") as ups_pool,
            tc.tile_pool(name="ssb", bufs=int(os.environ.get("SSB", 3))) as ssb_pool,
            tc.tile_pool(name="vps", bufs=int(os.environ.get("VPB", 1)), space="PSUM") as vps_pool,
            tc.tile_pool(name="vsb", bufs=int(os.environ.get("VSB", 3))) as vsb_pool,
            tc.tile_pool(name="yps", bufs=int(os.environ.get("YPB", 1)), space="PSUM") as yps_pool,
            tc.tile_pool(name="ysb", bufs=int(os.environ.get("YSB", 2))) as ysb_pool,
        ):
            wup2_sb = cpool.tile([128, 128], bf16)
            nc.sync.dma_start(out=wup2_sb[:], in_=wup2[:])
            wdn3_sb = cpool.tile([UP_LEN, W], bf16)
            nc.sync.dma_start(out=wdn3_sb[:], in_=wdn3[:])
            wdn2_sb = cpool.tile([128, 128], bf16)
            nc.sync.dma_start(out=wdn2_sb[:], in_=wdn2[:])
            alpha_sb = cpool.tile([128, 1], f32)
            nc.vector.memset(alpha_sb[:], NEG_SLOPE)

            import contextlib
            rep_ctx = (tc.For_i(0, repeat, 1) if repeat > 1
                       else contextlib.nullcontext())
            with rep_ctx:
                _pipelined_groups(nc, tc, mybir, n_groups, dma_b, locals())

    nc.compile()
    return nc


def _copy(nc, engine, out, in_):
    import concourse.mybir as mybir
    if engine == "act":
        nc.scalar.activation(out=out, in_=in_,
                             func=mybir.ActivationFunctionType.Copy)
    else:
        nc.vector.tensor_copy(out=out, in_=in_)


def _pipelined_groups(nc, tc, mybir, n_groups, dma_b, env):
    f32 = mybir.dt.float32
    bf16 = mybir.dt.bfloat16
    x, out = env["x"], env["out"]
    wup2_sb, wdn3_sb = env["wup2_sb"], env["wdn3_sb"]
    wdn2_sb, alpha_sb = env["wdn2_sb"], env["alpha_sb"]
    xpool, t1ps_pool, t1sb_pool = env["xpool"], env["t1ps_pool"], env["t1sb_pool"]
    ups_pool, ssb_pool, vps_pool = env["ups_pool"], env["ssb_pool"], env["vps_pool"]
    vsb_pool, yps_pool, ysb_pool = env["vsb_pool"], env["yps_pool"], env["ysb_pool"]

    state = {}          # per-group tiles passed between stages
    xt_cur = [None]
    ysb_cur = [None]

    def stage_a(g):
        gb = g % dma_b
        if gb == 0:
            xt_cur[0] = xpool.tile([64, dma_b * 512], bf16)
            nc.sync.dma_start(
                out=xt_cur[0][:], in_=x[:, g * 512:(g + dma_b) * 512])
        xt = xt_cur[0][:, gb * 512:(gb + 1) * 512]

        # M1: 4 MMs (K=64) -> t1 [128 (e,w), 512 (p4,h')] in one PSUM bank
        t1ps = t1ps_pool.tile([128, 512], f32)
        for p4 in range(4):
            nc.tensor.matmul(
                t1ps[:, p4 * 128:(p4 + 1) * 128],
                lhsT=xt[:, p4 * 128:(p4 + 1) * 128],
                rhs=wup2_sb[0:64, :],
                start=True, stop=True,
            )
        t1sb = t1sb_pool.tile([128, 512], bf16)
        _copy(nc, EVAC_T1, t1sb[:], t1ps[:])
        state[g] = {"t1sb": t1sb}

    def stage_b(g):
        st = state[g]
        t1sb = st["t1sb"]
        # M2: 2 row-tiled MMs (K=64, N=512) -> u [128 w', (e,p4,h')]
        ups = ups_pool.tile([128, 1024], f32)
        if M2_ROWTILE:
            for e in range(2):
                nc.tensor.matmul(
                    ups[:, e * 512:(e + 1) * 512],
                    lhsT=wup2_sb[e * 64:(e + 1) * 64, :],
                    rhs=t1sb[e * 64:(e + 1) * 64, :],
                    start=True, stop=True,
                )
        else:
            for e in range(2):
                for half in range(2):
                    nc.tensor.matmul(
                        ups[:, e * 512 + half * 256:e * 512 + (half + 1) * 256],
                        lhsT=wup2_sb[e * 64:(e + 1) * 64, :],
                        rhs=t1sb[e * 64:(e + 1) * 64,
                                 half * 256:(half + 1) * 256],
                        start=True, stop=True,
                    )
        ssb = ssb_pool.tile([128, 1024], bf16)
        nc.scalar.activation(
            out=ssb[:], in_=ups[:],
            func=mybir.ActivationFunctionType.Prelu,
            scale=1.0, alpha=alpha_sb[:],
        )
        st["ssb"] = ssb

    def stage_c(g):
        st = state.pop(g)
        ssb = st["ssb"]
        gb = g % dma_b
        if gb == 0:
            ysb_cur[0] = ysb_pool.tile([128, dma_b * 256], bf16)
        ysb4 = ysb_cur[0]

        # M3: 8 MMs -> v per plane [128 h', 64 w'']
        vps = vps_pool.tile([128, 512], f32)
        for p in range(GROUP):
            p4, e = p // 2, p % 2
            s_off = e * 512 + p4 * 128
            nc.tensor.matmul(
                vps[:, p * 64:(p + 1) * 64],
                lhsT=ssb[:, s_off:s_off + 128],
                rhs=wdn3_sb[:],
                start=True, stop=True,
            )
        vsb = vsb_pool.tile([128, 512], bf16)
        _copy(nc, EVAC_V, vsb[:], vps[:])

        # M4: 2 col-tiled MMs -> y [128 (ph,h''), 256 (pl,w'')]
        if Y_COLTILE:
            yps = yps_pool.tile([128, 256], f32)
            for ph in range(2):
                nc.tensor.matmul(
                    yps[ph * 64:(ph + 1) * 64, :],
                    lhsT=wdn2_sb[:, ph * 64:(ph + 1) * 64],
                    rhs=vsb[:, ph * 256:(ph + 1) * 256],
                    start=True, stop=True,
                )
            _copy(nc, EVAC_Y, ysb4[:, gb * 256:(gb + 1) * 256], yps[:])
        else:
            yps = yps_pool.tile([64, 512], f32)
            nc.tensor.matmul(
                yps[:], lhsT=wdn2_sb[:, 0:64], rhs=vsb[:],
                start=True, stop=True,
            )
            _copy(nc, EVAC_Y,
                  ysb4[0:64, gb * 256:(gb + 1) * 256], yps[:, 0:256])
            _copy(nc, EVAC_Y,
                  ysb4[64:128, gb * 256:(gb + 1) * 256], yps[:, 256:512])

        if gb == dma_b - 1:
            nc.sync.dma_start(
                out=out[:, (g - dma_b + 1) * 256:(g + 1) * 256],
                in_=ysb4[:],
            )

    # skewed emission: A(g+2), B(g+1), C(g)
    skew_b = int(os.environ.get("SKEW_B", 1))
    skew_total = int(os.environ.get("SKEW_C", 2))
    for i in range(n_groups + skew_total):
        ga = i
        gb_ = i - (skew_total - skew_b)
        gc = i - skew_total
        if ga < n_groups:
            stage_a(ga)
        if 0 <= gb_ < n_groups:
            stage_b(gb_)
        if 0 <= gc < n_groups:
            stage_c(gc)


def kernel(input, bias, up_filter, down_filter):
    global _LAST_RESULT, _CACHED
    from concourse.bass_utils import run_bass_kernel_spmd

    input = np.asarray(input, dtype=np.float32)
    bias = np.asarray(bias, dtype=np.float32)
    if np.any(bias):
        input = input + bias.reshape(1, C, 1, 1)

    if _CACHED is None:
        _CACHED = _build_bass()
    nc = _CACHED

    wup2_m, wdn3_m, wdn2_m = _build_consts(up_filter, down_filter)

    in_maps = []
    for i in range(N_CORES):
        in_maps.append({
            "x": _pack_x(input[i]),
            "wup2": wup2_m,
            "wdn3": wdn3_m,
            "wdn2": wdn2_m,
        })

    res = run_bass_kernel_spmd(nc, in_maps, core_ids=list(range(N_CORES)))
    _LAST_RESULT = res
    y = np.stack([_unpack_y(r["out"]) for r in res.results], axis=0)
    return np.ascontiguousarray(y)
